# revision 6
# baseline (speedup 1.0000x reference)
"""Class-routed autoencoder (moe_routing) Trainium2 kernel.

Strategy:
- The reference computes ALL 10 experts densely then gathers by label; we
  ROUTE instead: sort tokens by class on the host, split every class's tokens
  evenly across the 8 cores (class counts padded up to a multiple of 8 with
  dummy zero tokens), so every core runs an IDENTICAL program (SPMD) on
  N_core = sum_e ceil(c_e/8) tokens laid out as 10 contiguous single-class
  segments. Expert layers slice the right weight block per segment at
  compile time; no gather/scatter on device.
- Everything runs feature-major ([features, tokens]): weights are the
  stationary matmul operand as-is (out = W.T @ x_fm), the batch is the
  moving/free dim, and per-feature bias + ReLU + PSUM->SBUF evacuation fuse
  into one scalar-engine activation op (bias is per-partition).
- Matmuls run in bf16 (fp32 PSUM accumulate, biases added in fp32).
- enc1+enc2 and dec1+dec2 are fused per chunk so the big hidden activations
  (2048-dim) never leave SBUF; h2/e1/e2 (bottleneck dims) are SBUF-resident
  full width. Chunks: [512, rest split <=512] (PSUM bank limit).
- Startup is DMA-bound (first W1/x slices gate the PE). Scheme: W1 loads as
  8 contiguous K-slices (0.5MB, 4KB/partition rows); x chunk0 as two
  k-halves; triggers alternate sync/scalar HWDGE sequencers in consumption
  order so data arrives exactly as the PE needs it. enc1 of chunk 0 runs
  K-MAJOR in two m-phases (8 PSUM banks each) so 16 matmuls fire per
  arriving K-slice instead of waiting for all of W1. Bias loads go via
  gpsimd SWDGE (own queue). ~78 dummy warm-up matmuls on a memset tile keep
  the PE busy through the DMA wait so the HAM clock-gate is released
  (1.2->2.4GHz) and real matmuls start warm.
- W1/x tiles live in a RIGHT-side SBUF pool released after the last enc1
  chunk, so the decoder weights (gpsimd-triggered into that space) stream
  in during enc2/experts and are resident long before the decoder starts.
- Expert execution is interleaved into the encoder chunk loop: each expert
  runs as soon as the encoder chunks covering its column segment are done,
  so the expert-weight DMA stream (5-deep ring, issued with no false
  dependencies) hides entirely under encoder compute.
- Host: permute+transpose x, run 8 cores, inverse-permute the output.
"""

import ml_dtypes
import numpy as np

import concourse.bass as bass
import concourse.mybir as mybir
import concourse.tile as tile
from concourse import bacc
from concourse.bass_utils import run_bass_kernel_spmd

N_CORES = 8
N_CLS = 10
D_IN, D_H, D_BOT, D_EXP = 1024, 2048, 512, 1024

F32 = mybir.dt.float32
BF16 = mybir.dt.bfloat16
RELU = mybir.ActivationFunctionType.Relu
IDENT = mybir.ActivationFunctionType.Identity

CHUNK = 512  # max matmul moving-operand (free dim) size: one PSUM bank fp32
N_WARM = 68  # HAM warm-up matmuls (128 cols each) during startup DMA wait


def _chunks_of(n, step):
    """Balanced split of n into ceil(n/step) near-equal pieces (all <= step)."""
    nch = -(-n // step)
    base, rem = divmod(n, nch)
    out = []
    s = 0
    for i in range(nch):
        sz = base + (1 if i < rem else 0)
        out.append((s, sz))
        s += sz
    return out


def _chunks(n_core):
    """Chunk schedule: big first chunk (overlaps the DMA ramp), rest balanced."""
    c0 = min(CHUNK, n_core)
    out = [(0, c0)]
    if n_core > c0:
        out += [(c0 + s, sz) for s, sz in _chunks_of(n_core - c0, CHUNK)]
    return out


def _build(n_seg, n_core):
    """Build the SPMD program for per-class-per-core counts n_seg (sum=n_core)."""
    nc = bacc.Bacc()

    xt = nc.dram_tensor("xt", [D_IN, n_core], BF16, kind="ExternalInput")
    w1 = nc.dram_tensor("w1", [D_IN, D_H], BF16, kind="ExternalInput")
    b1 = nc.dram_tensor("b1", [128, D_H // 128], F32, kind="ExternalInput")
    w2 = nc.dram_tensor("w2", [D_H, D_BOT], BF16, kind="ExternalInput")
    b2 = nc.dram_tensor("b2", [128, D_BOT // 128], F32, kind="ExternalInput")
    ew1 = nc.dram_tensor("ew1", [N_CLS, D_BOT, D_EXP], BF16, kind="ExternalInput")
    eb1 = nc.dram_tensor("eb1", [128, N_CLS, D_EXP // 128], F32, kind="ExternalInput")
    ew2 = nc.dram_tensor("ew2", [N_CLS, D_EXP, D_BOT], BF16, kind="ExternalInput")
    eb2 = nc.dram_tensor("eb2", [128, N_CLS, D_BOT // 128], F32, kind="ExternalInput")
    dw1 = nc.dram_tensor("dw1", [D_BOT, D_H], BF16, kind="ExternalInput")
    db1 = nc.dram_tensor("db1", [128, D_H // 128], F32, kind="ExternalInput")
    dw2 = nc.dram_tensor("dw2", [D_H, D_IN], BF16, kind="ExternalInput")
    db2 = nc.dram_tensor("db2", [128, D_IN // 128], F32, kind="ExternalInput")
    out = nc.dram_tensor("out", [D_IN, n_core], F32, kind="ExternalOutput")

    segs = []  # (class e, col start, col len)
    s = 0
    for e in range(N_CLS):
        if n_seg[e] > 0:
            segs.append((e, s, n_seg[e]))
            s += n_seg[e]
    chunks = _chunks(n_core)

    KT1, MT1 = D_IN // 128, D_H // 128     # enc1: 8, 16
    KT2, MT2 = D_H // 128, D_BOT // 128    # enc2: 16, 4
    KE1, ME1 = D_BOT // 128, D_EXP // 128  # exp1: 4, 8
    KE2, ME2 = D_EXP // 128, D_BOT // 128  # exp2: 8, 4
    KD1, MD1 = D_BOT // 128, D_H // 128    # dec1: 4, 16
    KD2, MD2 = D_H // 128, D_IN // 128     # dec2: 16, 8

    with tile.TileContext(nc) as tc:
        p_const = tc.alloc_tile_pool(name="const", bufs=1)
        p_ps = tc.alloc_tile_pool(name="ps", bufs=8, space="PSUM")

        # ---- HAM warm-up: dummy matmuls on a zeroed tile keep the PE busy
        # through the startup DMA wait so the clock-gate releases to 2.4GHz
        # (takes ~3.4us of sustained activity) and real matmuls start warm.
        junk = p_const.tile([128, 128], BF16, tag="junk", name="junk")
        nc.vector.memset(junk[:], 0.0)
        for _ in range(N_WARM):
            wps = p_ps.tile([128, 128], F32, tag="ps", name="ps")
            nc.tensor.matmul(wps, junk[:], junk[:], start=True, stop=True)

        def bias_tile(h, tag, shape):
            # gpsimd SWDGE: tiny transfers, own trigger queue -> never on the
            # critical sync/scalar trigger path.
            t = p_const.tile(shape, F32, tag=tag, name=tag)
            nc.gpsimd.dma_start(out=t, in_=h[:])
            return t

        # bottleneck activations, SBUF-resident at full width
        p_e2 = tc.alloc_tile_pool(name="e2", bufs=1)
        p_h2 = tc.alloc_tile_pool(name="h2", bufs=1)
        e2_t = [p_e2.tile([128, n_core], BF16, tag=f"e2_{m}", name=f"e2_{m}")
                for m in range(D_BOT // 128)]
        h2_t = [p_h2.tile([128, n_core], BF16, tag=f"h2_{m}", name=f"h2_{m}")
                for m in range(D_BOT // 128)]

        # Expert pool is allocated BEFORE the encoder pools: its space never
        # overlaps encoder tiles, so expert-weight DMAs carry no false deps
        # and prefetch during the encoder phase.
        EW_BUFS = 5
        ECHUNK = 256
        p_exp = tc.alloc_tile_pool(name="exp", bufs=1)
        e1_ring = [p_exp.tile([128, D_EXP // 128, ECHUNK], BF16, tag=f"e1r_{i}",
                              name=f"e1r_{i}") for i in range(3)]
        ew1_ring = [p_exp.tile([128, KE1, D_EXP], BF16, tag=f"ew1_{i}",
                               name=f"ew1_{i}") for i in range(EW_BUFS)]
        ew2_ring = [p_exp.tile([128, KE2, D_BOT], BF16, tag=f"ew2_{i}",
                               name=f"ew2_{i}") for i in range(EW_BUFS)]

        # Long-lived encoder tiles (w2, h1c): left side. Short-lived startup
        # tiles (w1, x chunks): RIGHT side, released after the last enc1 so
        # the decoder weights can stream into that space during enc2/experts.
        p_encB = tc.alloc_tile_pool(name="encB", bufs=1)
        p_encA = tc.alloc_tile_pool(name="encA", bufs=1, side="right")

        c0_start, c0_len = chunks[0]

        # x chunk 0 as 8 per-K-tile slices (0.125MB each) and W1 as 16
        # HALF-K-slices [128 rows, 1024 cols] (0.25MB, 2KB/partition
        # contiguous): the K-major phase over m 0..7 consumes exactly one
        # (w1 half, x slice) pair per K step, so the startup demand rate
        # stays below DMA supply and the PE never starves after the first
        # pair lands.
        xk = [p_encA.tile([128, CHUNK], BF16, tag=f"xk{j}", name=f"xk{j}")
              for j in range(KT1)]
        w1h = [[p_encA.tile([128, D_H // 2], BF16, tag=f"w1h{j}_{h}",
                            name=f"w1h{j}_{h}") for h in range(2)]
               for j in range(KT1)]
        xc_rest = {}
        for ci in range(1, len(chunks)):
            xc_rest[ci] = p_encA.tile([128, KT1, CHUNK], BF16, tag="xc",
                                      name="xc", bufs=max(1, len(chunks) - 1))
        w2_tiles = [p_encB.tile([128, KT2 // 2, D_BOT], BF16, tag=f"w2_{i}",
                                name=f"w2_{i}") for i in range(2)]

        def xk_dma(eng, j):
            eng.dma_start(out=xk[j][:, :c0_len],
                          in_=xt[j * 128:(j + 1) * 128,
                                 c0_start:c0_start + c0_len])

        def w1h_dma(eng, j, h):
            eng.dma_start(out=w1h[j][h],
                          in_=w1[j * 128:(j + 1) * 128,
                                 h * (D_H // 2):(h + 1) * (D_H // 2)])

        # Two HWDGE rings (sync + scalar) round-robin at the DMA engines, so
        # alternating the emission makes data land in PE consumption order:
        # (w1h0a,xk0), (w1h1a,xk1), ..., then the m8..15 W1 halves, then w2
        # on sync while xc1/xc2 go on scalar.
        for j in range(KT1):
            w1h_dma(nc.sync if j % 2 == 0 else nc.scalar, j, 0)
            xk_dma(nc.scalar if j % 2 == 0 else nc.sync, j)
        for j in range(KT1):
            w1h_dma(nc.sync if j % 2 == 0 else nc.scalar, j, 1)
        for i in range(2):
            nc.sync.dma_start(
                out=w2_tiles[i],
                in_=w2[i * 8 * 128:(i + 1) * 8 * 128, :]
                .rearrange("(a p) n -> p a n", p=128))
        for ci in range(1, len(chunks)):
            cs, cl = chunks[ci]
            nc.scalar.dma_start(
                out=xc_rest[ci][:, :, :cl],
                in_=xt[:, cs:cs + cl].rearrange("(a p) n -> p a n", p=128))

        def w1_at(k, m):
            return w1h[k][m // 8][:, (m % 8) * 128:(m % 8 + 1) * 128]

        def w2_at(k):
            return w2_tiles[k // 8][:, k % 8, :]

        def xc0_at(k):
            return xk[k]

        # biases via gpsimd SWDGE, in order of first use
        b1_t = bias_tile(b1, "b1", [128, MT1])
        b2_t = bias_tile(b2, "b2", [128, MT2])
        eb1_t = bias_tile(eb1, "eb1", [128, N_CLS, ME1])
        eb2_t = bias_tile(eb2, "eb2", [128, N_CLS, ME2])
        db1_t = bias_tile(db1, "db1", [128, MD1])
        db2_t = bias_tile(db2, "db2", [128, MD2])

        # experts are emitted as soon as the encoder chunks covering their
        # column segment are done: their compute absorbs expert-weight DMA
        # latency, and the PE never waits on the weight stream at phase end.
        seg_queue = list(segs)
        exp_counter = [0]
        unit_ctr = [0]
        pend = [None]  # exp2 of each unit is delayed one unit behind its exp1

        def emit_exp1(u):
            e, a, al, slot, ew1_t, _ = u
            e1c = e1_ring[slot]
            for m in range(ME1):
                ps = p_ps.tile([128, al], F32, tag="ps", name="ps")
                for k in range(KE1):
                    nc.tensor.matmul(ps, ew1_t[:, k, m * 128:(m + 1) * 128],
                                     h2_t[k][:, a:a + al],
                                     start=(k == 0), stop=(k == KE1 - 1))
                # bias+relu on the (idle) vector engine: keeps PSUM
                # evacuation off the scalar engine's critical path
                nc.vector.tensor_scalar(
                    out=e1c[:, m, :al], in0=ps,
                    scalar1=eb1_t[:, e, m:m + 1], scalar2=0.0,
                    op0=mybir.AluOpType.add, op1=mybir.AluOpType.max)

        def emit_exp2(u):
            e, a, al, slot, _, ew2_t = u
            e1c = e1_ring[slot]
            for m in range(ME2):
                ps = p_ps.tile([128, al], F32, tag="ps", name="ps")
                for k in range(KE2):
                    nc.tensor.matmul(ps, ew2_t[:, k, m * 128:(m + 1) * 128],
                                     e1c[:, k, :al],
                                     start=(k == 0), stop=(k == KE2 - 1))
                nc.scalar.activation(out=e2_t[m][:, a:a + al], in_=ps,
                                     func=RELU, bias=eb2_t[:, e, m:m + 1],
                                     scale=1.0)

        def emit_expert(e, s0, sl):
            # exp1(unit i) then exp2(unit i-1): exp1's PSUM evacuations (DVE)
            # overlap the next unit's exp1 matmuls instead of stalling the PE
            ei = exp_counter[0]
            exp_counter[0] += 1
            ew1_t = ew1_ring[ei % EW_BUFS]
            nc.sync.dma_start(
                out=ew1_t, in_=ew1[e].rearrange("(a p) n -> p a n", p=128))
            ew2_t = ew2_ring[ei % EW_BUFS]
            nc.sync.dma_start(
                out=ew2_t, in_=ew2[e].rearrange("(a p) n -> p a n", p=128))
            for c0, cl in _chunks_of(sl, ECHUNK):
                u = (e, s0 + c0, cl, unit_ctr[0] % 3, ew1_t, ew2_t)
                unit_ctr[0] += 1
                emit_exp1(u)
                if pend[0] is not None:
                    emit_exp2(pend[0])
                pend[0] = u

        # ---------------- encoder (fused enc1+enc2 per chunk) -----------------
        p_decW = None
        dw1_tile = None
        dw2_tiles = None

        for ci, (c0, cl) in enumerate(chunks):
            h1c = []
            if ci == 0:
                # K-major in two m-phases: 16 matmuls fire per arriving
                # W1 K-slice; 8 PSUM banks accumulate across the K-loop.
                for half in range(2):
                    ms = list(range(half * 8, half * 8 + 8))
                    pss = {m: p_ps.tile([128, cl], F32, tag="ps", name="ps")
                           for m in ms}
                    for k in range(KT1):
                        for m in ms:
                            nc.tensor.matmul(pss[m], w1_at(k, m),
                                             xc0_at(k)[:, :cl],
                                             start=(k == 0),
                                             stop=(k == KT1 - 1))
                    for m in ms:
                        h = p_encB.tile([128, CHUNK], BF16, tag="h1c",
                                        name="h1c", bufs=MT1)
                        nc.scalar.activation(out=h[:, :cl], in_=pss[m],
                                             func=RELU, bias=b1_t[:, m:m + 1],
                                             scale=1.0)
                        h1c.append(h)
            else:
                for m in range(MT1):
                    ps = p_ps.tile([128, cl], F32, tag="ps", name="ps")
                    for k in range(KT1):
                        nc.tensor.matmul(ps, w1_at(k, m),
                                         xc_rest[ci][:, k, :cl],
                                         start=(k == 0), stop=(k == KT1 - 1))
                    h = p_encB.tile([128, CHUNK], BF16, tag="h1c", name="h1c",
                                    bufs=MT1)
                    nc.scalar.activation(out=h[:, :cl], in_=ps, func=RELU,
                                         bias=b1_t[:, m:m + 1], scale=1.0)
                    h1c.append(h)

            if ci == len(chunks) - 1:
                # w1/x space is dead after the last enc1: release it and
                # stream the decoder weights into it (gpsimd SWDGE) while
                # enc2/experts still run.
                p_encA.release()
                p_decW = tc.alloc_tile_pool(name="decW", bufs=1, side="right")
                dw1_tile = p_decW.tile([128, KD1, D_H], BF16, tag="dw1",
                                       name="dw1")
                nc.gpsimd.dma_start(
                    out=dw1_tile,
                    in_=dw1[:].rearrange("(a p) n -> p a n", p=128))
                dw2_tiles = []
                for i in range(2):
                    t = p_decW.tile([128, KD2 // 2, D_IN], BF16,
                                    tag=f"dw2_{i}", name=f"dw2_{i}")
                    nc.gpsimd.dma_start(
                        out=t,
                        in_=dw2[i * 8 * 128:(i + 1) * 8 * 128, :]
                        .rearrange("(a p) n -> p a n", p=128))
                    dw2_tiles.append(t)

            for m in range(MT2):
                ps = p_ps.tile([128, cl], F32, tag="ps", name="ps")
                for k in range(KT2):
                    nc.tensor.matmul(ps, w2_at(k)[:, m * 128:(m + 1) * 128],
                                     h1c[k][:, :cl],
                                     start=(k == 0), stop=(k == KT2 - 1))
                nc.scalar.activation(out=h2_t[m][:, c0:c0 + cl], func=RELU,
                                     in_=ps, bias=b2_t[:, m:m + 1], scale=1.0)
            # run every expert whose segment is fully covered by done chunks
            chunk_end = c0 + cl
            while seg_queue and seg_queue[0][1] + seg_queue[0][2] <= chunk_end:
                e, s0, sl = seg_queue.pop(0)
                emit_expert(e, s0, sl)

        for e, s0, sl in seg_queue:
            emit_expert(e, s0, sl)
        if pend[0] is not None:
            emit_exp2(pend[0])
            pend[0] = None

        dw1_at = lambda k: dw1_tile[:, k, :]
        dw2_at = lambda k: dw2_tiles[k // 8][:, k % 8, :]

        # d1c/o tiles go where w2/h1c were
        p_encB.release()
        p_decB = tc.alloc_tile_pool(name="decB", bufs=1)

        # ---------------- decoder (fused dec1+dec2 per chunk) -----------------
        for ci, (c0, cl) in enumerate(chunks):
            d1c = []
            for m in range(MD1):
                ps = p_ps.tile([128, cl], F32, tag="ps", name="ps")
                for k in range(KD1):
                    nc.tensor.matmul(ps, dw1_at(k)[:, m * 128:(m + 1) * 128],
                                     e2_t[k][:, c0:c0 + cl],
                                     start=(k == 0), stop=(k == KD1 - 1))
                d = p_decB.tile([128, CHUNK], BF16, tag="d1c", name="d1c",
                                bufs=MD1)
                nc.scalar.activation(out=d[:, :cl], in_=ps, func=RELU,
                                     bias=db1_t[:, m:m + 1], scale=1.0)
                d1c.append(d)
            for m in range(MD2):
                ps = p_ps.tile([128, cl], F32, tag="ps", name="ps")
                for k in range(KD2):
                    nc.tensor.matmul(ps, dw2_at(k)[:, m * 128:(m + 1) * 128],
                                     d1c[k][:, :cl],
                                     start=(k == 0), stop=(k == KD2 - 1))
                o_t = p_decB.tile([128, CHUNK], F32, tag="o", name="o", bufs=4)
                last = ci == len(chunks) - 1 and m == MD2 - 1
                if not last:
                    nc.scalar.activation(out=o_t[:, :cl], in_=ps, func=IDENT,
                                         bias=db2_t[:, m:m + 1], scale=1.0)
                    nc.sync.dma_start(
                        out=out[m * 128:(m + 1) * 128, c0:c0 + cl],
                        in_=o_t[:, :cl])
                else:
                    # final output tile: two half-width act+DMA chains on the
                    # two HWDGE sequencers so the drain after the last matmul
                    # is halved.
                    hl = cl // 2
                    nc.scalar.activation(out=o_t[:, :hl], in_=ps[:, :hl],
                                         func=IDENT,
                                         bias=db2_t[:, m:m + 1], scale=1.0)
                    nc.sync.dma_start(
                        out=out[m * 128:(m + 1) * 128, c0:c0 + hl],
                        in_=o_t[:, :hl])
                    nc.scalar.activation(out=o_t[:, hl:cl], in_=ps[:, hl:cl],
                                         func=IDENT,
                                         bias=db2_t[:, m:m + 1], scale=1.0)
                    nc.scalar.dma_start(
                        out=out[m * 128:(m + 1) * 128, c0 + hl:c0 + cl],
                        in_=o_t[:, hl:cl])

        p_decB.release()
        p_decW.release()
        p_exp.release()
        p_h2.release()
        p_e2.release()
        p_ps.release()
        p_const.release()

    nc.finalize()
    return nc


_CACHE = {}


def _get_nc(n_seg, n_core):
    key = tuple(n_seg)
    if key not in _CACHE:
        _CACHE[key] = _build(n_seg, n_core)
    return _CACHE[key]


def _bf16(a):
    return np.ascontiguousarray(np.asarray(a, np.float32).astype(ml_dtypes.bfloat16))


def _bias_fm(b, mt):
    """[mt*128] -> [128, mt] feature-major (partition-contiguous) layout."""
    return np.ascontiguousarray(np.asarray(b, np.float32).reshape(mt, 128).T)


def _ebias_fm(b, mt):
    """[N_CLS, mt*128] -> [128, N_CLS, mt]."""
    a = np.asarray(b, np.float32).reshape(N_CLS, mt, 128)
    return np.ascontiguousarray(a.transpose(2, 0, 1))


def kernel(x, labels, W1, b1, W2, b2, EW1, Eb1, EW2, Eb2, DW1, Db1, DW2, Db2):
    x = np.asarray(x, dtype=np.float32)
    labels_np = np.asarray(labels).astype(np.int64)
    B = x.shape[0]

    counts = np.bincount(labels_np, minlength=N_CLS)
    n_seg = [int(-(-int(c) // N_CORES)) for c in counts]  # ceil(c/8)
    n_core = int(sum(n_seg))

    # assign tokens: class e sorted tokens padded to 8*n_seg[e], row j -> core j
    order = np.argsort(labels_np, kind="stable")
    idx_by_class = np.split(order, np.cumsum(counts)[:-1])
    core_tok = np.full((N_CORES, n_core), -1, dtype=np.int64)
    off = 0
    for e in range(N_CLS):
        ne = n_seg[e]
        if ne == 0:
            continue
        padded = np.full(N_CORES * ne, -1, dtype=np.int64)
        padded[:counts[e]] = idx_by_class[e]
        core_tok[:, off:off + ne] = padded.reshape(N_CORES, ne)
        off += ne

    weights = {
        "w1": _bf16(W1), "b1": _bias_fm(b1, D_H // 128),
        "w2": _bf16(W2), "b2": _bias_fm(b2, D_BOT // 128),
        "ew1": _bf16(EW1), "eb1": _ebias_fm(Eb1, D_EXP // 128),
        "ew2": _bf16(EW2), "eb2": _ebias_fm(Eb2, D_BOT // 128),
        "dw1": _bf16(DW1), "db1": _bias_fm(Db1, D_H // 128),
        "dw2": _bf16(DW2), "db2": _bias_fm(Db2, D_IN // 128),
    }

    x_bf = x.astype(ml_dtypes.bfloat16)
    in_maps = []
    for j in range(N_CORES):
        ids = core_tok[j]
        valid = ids >= 0
        xc = np.zeros((n_core, D_IN), dtype=ml_dtypes.bfloat16)
        xc[valid] = x_bf[ids[valid]]
        im = {"xt": np.ascontiguousarray(xc.T)}
        im.update(weights)
        in_maps.append(im)

    nc = _get_nc(n_seg, n_core)
    res = run_bass_kernel_spmd(nc, in_maps, core_ids=list(range(N_CORES)))

    out = np.empty((B, D_IN), dtype=np.float32)
    for j in range(N_CORES):
        oc = res.results[j]["out"]  # [D_IN, n_core]
        ids = core_tok[j]
        valid = ids >= 0
        out[ids[valid]] = oc.T[valid]
    return out
